# revision 2
# baseline (speedup 1.0000x reference)
"""Chunked attention kernel for Trainium2 (Bass/Tile), SPMD over 8 NeuronCores.

Problem (hardcoded):
  x: [B=8, C=1024, L=4096] fp32, Wq/Wk/Wv/Wo: [1024,1024] fp32 (stored [in,out]),
  biases [1024] fp32.  H=8 heads, head_dim=128, CHUNK=64 (block-diagonal attention).
  out = transpose(softmax((xt@Wq)(xt@Wk)^T/sqrt(128) blockwise) @ (xt@Wv) @ Wo, [B,C,L])

Sharding: data-parallel over B - one batch per core. No collectives.

Numerics: the four C x C projections run as fp8(e4m3) DoubleRow matmuls.
DoubleRow packs TWO 128-deep contraction tiles per PE instruction at 0.5
cycles/output-row (4x fp16 matmul throughput). To keep accuracy, every
projection input is split hi/lo into two e4m3 tensors (value = hi + lo,
representation error ~1.3e-3) and each projection computes the three
significant bilinear terms hi*Whi + lo*Whi + hi*Wlo (the lo*Wlo term is
~1e-3 relative and dropped): 12 DoubleRow ops per [128,512] output tile
vs 16 packed tiles, i.e. 3072 PE cycles vs 4096 for fp16. Tensors are
pre-scaled (x*16, W*256) so e4m3's 3-bit mantissa sees values well inside
its normal range; descales fold into existing eviction copies for free.

The softmax denominator (column sums of exp, replicated over partitions)
moves off the PE onto the Pool engine via partition_all_reduce, removing
the ones-matmul entirely.

Per-core dataflow (all projections fp8 DoubleRow / fp32 PSUM):
  Q^T[c,l], K^T[c,l]  (feature-major)  V[l,c] (token-major)
  per head h, chunk-pair p: S^T[k,q] = K^T block x Q^T block (fp16)
    E = exp(S^T/sqrt(128)) on diagonal 64x64 blocks (ACT), rest zero
    D = partition_all_reduce(E) on Pool; EN = E/D (DVE)
    P^T[d,q] = V block x EN (fp16); split 16*P hi/lo to fp8 (Pool)
  out^T[c,l] = Wo^T P^T as 3-term fp8 DoubleRow
"""

import numpy as np
from contextlib import ExitStack

import concourse.bass as bass
import concourse.bacc as bacc
import concourse.tile as tile
import concourse.mybir as mybir
from concourse import bass_isa

B, C, L = 8, 1024, 4096
H, HD, CHUNK, PAIR = 8, 128, 64, 128
N_CORES = 8
KT = C // 128          # 8 contraction tiles
NJP = KT // 2          # DoubleRow j-pairs per term
LT = 512               # tokens per strip
F8 = mybir.dt.float8e4
F16 = mybir.dt.float16
F32 = mybir.dt.float32
NP8 = mybir.dt.np(F8)
SCALE = 1.0 / float(np.sqrt(HD))
DRMODE = mybir.MatmulPerfMode.DoubleRow
SX = 16.0              # x pre-scale into e4m3 range
SW = 256.0             # weight pre-scale (sigma 1/32 -> 8)
SP = 16.0              # attention-output pre-scale
QSCALE = 1.0 / (SX * SW)   # Q/K/V eviction descale
OSCALE = 1.0 / (SP * SW)   # out eviction descale
WNAMES = ("wq", "wk", "wv", "wo")


def _emit(ctx, tc, xh_d, xl_d, w_d, o_d, l_total):
    nc = tc.nc
    NS = l_total // LT     # strips
    NP = LT // PAIR        # chunk-pairs (= token 128-tiles) per strip

    wpool = ctx.enter_context(tc.tile_pool(name="w", bufs=1))
    xpool = ctx.enter_context(tc.tile_pool(name="xp", bufs=2))
    qpool = ctx.enter_context(tc.tile_pool(name="qp", bufs=2))
    vpool = ctx.enter_context(tc.tile_pool(name="vp", bufs=2))
    epool = ctx.enter_context(tc.tile_pool(name="ep", bufs=1))
    rpool = ctx.enter_context(tc.tile_pool(name="rp", bufs=2))
    dpool = ctx.enter_context(tc.tile_pool(name="dp", bufs=2))
    npool = ctx.enter_context(tc.tile_pool(name="np", bufs=2))
    tpool = ctx.enter_context(tc.tile_pool(name="tp", bufs=2))
    p16pool = ctx.enter_context(tc.tile_pool(name="p16", bufs=2))
    p8pool = ctx.enter_context(tc.tile_pool(name="p8", bufs=2))
    opool = ctx.enter_context(tc.tile_pool(name="op", bufs=4))
    pjps = ctx.enter_context(tc.tile_pool(name="pj", bufs=4, space="PSUM"))
    scps = ctx.enter_context(tc.tile_pool(name="sc", bufs=2, space="PSUM"))
    pvps = ctx.enter_context(tc.tile_pool(name="pv", bufs=2, space="PSUM"))

    # --- persistent fp8 weights: [128, KT*C] per (tensor, hi/lo); k-tile j
    # --- lives at cols [j*C, (j+1)*C). DoubleRow views are [128, j, C].
    wtile, wview = {}, {}
    for n in WNAMES:
        for part in ("h", "l"):
            t = wpool.tile([128, KT * C], F8, tag=f"{n}{part}")
            wtile[(n, part)] = t
            wview[(n, part)] = t.rearrange("p (j c) -> p j c", c=C)

    def dma_w(n, part, j):
        nc.sync.dma_start(wtile[(n, part)][:, j * C:(j + 1) * C],
                          w_d[n + part][j * 128:(j + 1) * 128, :])

    def load_x(s):
        th = xpool.tile([128, KT * LT], F8, tag="xh")
        tl = xpool.tile([128, KT * LT], F8, tag="xl")
        for j in range(KT):
            nc.sync.dma_start(th[:, j * LT:(j + 1) * LT],
                              xh_d[j * 128:(j + 1) * 128, s * LT:(s + 1) * LT])
            nc.sync.dma_start(tl[:, j * LT:(j + 1) * LT],
                              xl_d[j * 128:(j + 1) * 128, s * LT:(s + 1) * LT])
        return th, tl

    # Startup DMA order follows first-use order: strip-0 x (hi+lo) and wq-hi
    # interleaved (enables the hi*Whi / lo*Whi terms of every head's Q), then
    # wq-lo + wk-hi, wk-lo, then V/O weights which are only needed later.
    xh0 = xpool.tile([128, KT * LT], F8, tag="xh")
    xl0 = xpool.tile([128, KT * LT], F8, tag="xl")
    for j in range(KT):
        nc.sync.dma_start(xh0[:, j * LT:(j + 1) * LT],
                          xh_d[j * 128:(j + 1) * 128, 0:LT])
        nc.sync.dma_start(xl0[:, j * LT:(j + 1) * LT],
                          xl_d[j * 128:(j + 1) * 128, 0:LT])
        dma_w("wq", "h", j)
    for j in range(KT):
        dma_w("wq", "l", j)
        dma_w("wk", "h", j)
    for j in range(KT):
        dma_w("wk", "l", j)
    for j in range(KT):
        dma_w("wv", "h", j)
        dma_w("wv", "l", j)
    for j in range(KT):
        dma_w("wo", "h", j)
        dma_w("wo", "l", j)
    x_next = (xh0, xl0)

    # e_t is a single persistent buffer: exps rewrite the diagonal blocks every
    # strip, the off-diagonal stays zero from this one memset.
    e_t = epool.tile([128, H * LT], F16, tag="e")
    nc.gpsimd.memset(e_t[:], 0.0)

    def dr3(ps, pairs, lcols, rcols):
        """12 DoubleRow matmuls: 3 bilinear terms x 4 j-pairs into one psum.
        pairs = ((rhs_hi, lhs_hi), (rhs_lo, lhs_hi), (rhs_hi, lhs_lo)) views
        [128, j, *]; lcols/rcols slice the stationary/moving free columns."""
        n = 0
        for rv, lv in pairs:
            for jp in range(NJP):
                n += 1
                nc.tensor.matmul(ps,
                                 lv[:, 2 * jp:2 * jp + 2, lcols],
                                 rv[:, 2 * jp:2 * jp + 2, rcols],
                                 start=(n == 1), stop=(n == 3 * NJP),
                                 perf_mode=DRMODE)

    def o_proj(ph_v, pl_v, ls, final):
        woh, wol = wview[("wo", "h")], wview[("wo", "l")]
        for m in range(KT):
            ps = pjps.tile([128, 512], F32, tag="pj")
            dr3(ps[:, 0:LT],
                ((ph_v, woh), (pl_v, woh), (ph_v, wol)),
                slice(m * 128, (m + 1) * 128), slice(None))
            if final and m == KT - 1:
                # last output block in quarters so the post-PE drain is short
                for q in range(4):
                    o_t = opool.tile([128, 128], F32, tag="oq")
                    nc.vector.tensor_scalar_mul(o_t[:], ps[:, q * 128:(q + 1) * 128],
                                                OSCALE)
                    nc.sync.dma_start(
                        o_d[m * 128:(m + 1) * 128, ls + q * 128:ls + (q + 1) * 128],
                        o_t[:])
            else:
                o_t = opool.tile([128, LT], F32, tag="o")
                nc.vector.tensor_scalar_mul(o_t[:], ps[:, 0:LT], OSCALE)
                nc.sync.dma_start(o_d[m * 128:(m + 1) * 128, ls:ls + LT],
                                  o_t[:, 0:LT])

    for s in range(NS):
        ls = s * LT
        xh_t, xl_t = x_next if s == 0 else load_x(s)
        xh_v = xh_t.rearrange("p (j n) -> p j n", n=LT)
        xl_v = xl_t.rearrange("p (j n) -> p j n", n=LT)

        qk_t = qpool.tile([128, 2 * KT * LT], F16, tag="qk")
        en_t = npool.tile([128, H * LT], F16, tag="en")

        # --- Q/K projections (fp8 DoubleRow) + per-head scores/softmax
        for h in range(H):
            qb = h * 2 * LT           # Q cols for head h
            kb = h * 2 * LT + LT      # K cols for head h
            for off, nm in ((qb, "wq"), (kb, "wk")):
                ps = pjps.tile([128, 512], F32, tag="pj")
                wh, wl = wview[(nm, "h")], wview[(nm, "l")]
                dr3(ps[:, 0:LT], ((xh_v, wh), (xl_v, wh), (xh_v, wl)),
                    slice(h * 128, (h + 1) * 128), slice(None))
                nc.vector.tensor_scalar_mul(qk_t[:, off:off + LT], ps[:, 0:LT],
                                            QSCALE)
            sc = scps.tile([128, LT], F32, tag="sc")
            for p in range(NP):
                nc.tensor.matmul(sc[:, p * PAIR:(p + 1) * PAIR],
                                 qk_t[:, kb + p * PAIR:kb + (p + 1) * PAIR],
                                 qk_t[:, qb + p * PAIR:qb + (p + 1) * PAIR],
                                 start=True, stop=True)
            # exp of the diagonal 64x64 blocks of every pair -> e_t (off-diag
            # stays 0). One strided ACT per half: [64, (pairs), 64] pattern.
            eh = e_t[:, h * LT:(h + 1) * LT]
            for r0, c0 in ((0, 0), (64, 64)):
                nc.scalar.activation(
                    eh[r0:r0 + 64, :].rearrange("a (np c) -> a np c", c=PAIR)[:, :, c0:c0 + 64],
                    sc[r0:r0 + 64, :].rearrange("a (np c) -> a np c", c=PAIR)[:, :, c0:c0 + 64],
                    mybir.ActivationFunctionType.Exp, scale=SCALE)
            # softmax denominator: Pool all-reduce over partitions replicates
            # the per-column sum on every partition (off-diag zeros are inert);
            # reciprocal + normalize on DVE.
            den = dpool.tile([128, LT], F32, tag="den")
            nc.gpsimd.partition_all_reduce(den[:], eh, channels=128,
                                           reduce_op=bass_isa.ReduceOp.add)
            r_t = rpool.tile([128, LT], F16, tag="r")
            with nc.allow_low_precision(reason="softmax recip fp16 ample"):
                nc.vector.reciprocal(r_t[:], den[:])
            nc.vector.tensor_mul(en_t[:, h * LT:(h + 1) * LT], eh, r_t[:])

        # --- V projection (token-major, fp8 DoubleRow): V[l, c]
        v_t = vpool.tile([128, NP * C], F16, tag="v")
        wvh, wvl = wview[("wv", "h")], wview[("wv", "l")]
        for p in range(NP):
            for n2 in range(C // LT):
                ps = pjps.tile([128, 512], F32, tag="pj")
                dr3(ps[:], ((wvh, xh_v), (wvh, xl_v), (wvl, xh_v)),
                    slice(p * 128, (p + 1) * 128), slice(n2 * LT, (n2 + 1) * LT))
                nc.vector.tensor_scalar_mul(
                    v_t[:, p * C + n2 * LT:p * C + (n2 + 1) * LT], ps[:], QSCALE)

        # --- attention output P^T[d, q] per head, evict 16*P to fp16, then
        # --- split hi/lo to fp8 for the DoubleRow output projection. The
        # --- split runs on Pool (idle) except the last strip, where the
        # --- sub/quant half moves to DVE so Pool is off the critical tail.
        ph8_t = p8pool.tile([128, KT * LT], F8, tag="ph8")
        pl8_t = p8pool.tile([128, KT * LT], F8, tag="pl8")
        for h in range(H):
            pv = pvps.tile([128, NP * PAIR], F32, tag="pv")
            for p in range(NP):
                nc.tensor.matmul(pv[:, p * PAIR:(p + 1) * PAIR],
                                 v_t[:, p * C + h * 128:p * C + (h + 1) * 128],
                                 en_t[:, h * LT + p * PAIR:h * LT + (p + 1) * PAIR],
                                 start=True, stop=True)
            p16 = p16pool.tile([128, LT], F16, tag="p16")
            nc.vector.tensor_scalar_mul(p16[:], pv[:], SP)
            hs = slice(h * LT, (h + 1) * LT)
            t16 = tpool.tile([128, LT], F16, tag="t16")
            l16 = tpool.tile([128, LT], F16, tag="l16")
            eng2 = nc.vector if s == NS - 1 else nc.gpsimd
            nc.gpsimd.tensor_copy(ph8_t[:, hs], p16[:])
            nc.gpsimd.tensor_copy(t16[:], ph8_t[:, hs])
            eng2.tensor_sub(l16[:], p16[:], t16[:])
            eng2.tensor_copy(pl8_t[:, hs], l16[:])

        # --- output projection of the PREVIOUS strip (software pipelining)
        if s >= 1:
            o_proj(*p_prev, ls_prev, final=False)
        p_prev = (ph8_t.rearrange("p (j n) -> p j n", n=LT),
                  pl8_t.rearrange("p (j n) -> p j n", n=LT))
        ls_prev = ls
    o_proj(*p_prev, ls_prev, final=True)


def build_nc(l_total=L):
    nc = bacc.Bacc("TRN2", target_bir_lowering=False, debug=False,
                   enable_asserts=False)
    xh_d = nc.dram_tensor("xh", [C, l_total], F8, kind="ExternalInput").ap()
    xl_d = nc.dram_tensor("xl", [C, l_total], F8, kind="ExternalInput").ap()
    w_d = {}
    for n in WNAMES:
        for part in ("h", "l"):
            w_d[n + part] = nc.dram_tensor(n + part, [C, C], F8,
                                           kind="ExternalInput").ap()
    o_d = nc.dram_tensor("out", [C, l_total], F32, kind="ExternalOutput").ap()
    with tile.TileContext(nc) as tc:
        with ExitStack() as ctx:
            _emit(ctx, tc, xh_d, xl_d, w_d, o_d, l_total)
    nc.compile()
    return nc


_NC_CACHE = {}


def _get_nc(l_total):
    if l_total not in _NC_CACHE:
        _NC_CACHE[l_total] = build_nc(l_total)
    return _NC_CACHE[l_total]


def _split8(a, scale):
    s = np.asarray(a, np.float32) * scale
    hi = s.astype(NP8)
    lo = (s - hi.astype(np.float32)).astype(NP8)
    return np.ascontiguousarray(hi), np.ascontiguousarray(lo)


def make_in_maps(x, Wq, Wk, Wv, Wo):
    ws = {}
    for n, w in zip(WNAMES, (Wq, Wk, Wv, Wo)):
        ws[n + "h"], ws[n + "l"] = _split8(w, SW)
    xs = np.asarray(x, np.float32)
    in_maps = []
    for i in range(x.shape[0]):
        xh, xl = _split8(xs[i], SX)
        m = {"xh": xh, "xl": xl}
        m.update(ws)
        in_maps.append(m)
    return in_maps


def _numpy_fallback(x, Wq, bq, Wk, bk, Wv, bv, Wo, bo):
    # Exact host-side path, used only if biases are nonzero (the problem spec
    # fills them with zeros, so the device kernel does not apply them).
    x = np.asarray(x, np.float32)
    Bn, Cn, Ln = x.shape
    hd = Cn // H
    nch = Ln // CHUNK
    xt = np.transpose(x, (0, 2, 1))
    Q = (xt @ Wq + bq).reshape(Bn, nch, CHUNK, H, hd)
    K = (xt @ Wk + bk).reshape(Bn, nch, CHUNK, H, hd)
    V = (xt @ Wv + bv).reshape(Bn, nch, CHUNK, H, hd)
    scores = np.einsum("bnqhd,bnkhd->bnhqk", Q, K) / np.sqrt(hd)
    scores -= scores.max(axis=-1, keepdims=True)
    e = np.exp(scores)
    attn = e / e.sum(axis=-1, keepdims=True)
    out = np.einsum("bnhqk,bnkhd->bnqhd", attn, V).reshape(Bn, Ln, Cn)
    out = out @ Wo + bo
    return np.ascontiguousarray(np.transpose(out, (0, 2, 1)).astype(np.float32))


def kernel(x, Wq, bq, Wk, bk, Wv, bv, Wo, bo, trace=False):
    from concourse.bass_utils import run_bass_kernel_spmd
    nb, c_in, l_total = x.shape
    if (any(np.any(np.asarray(b) != 0) for b in (bq, bk, bv, bo))
            or c_in != C or l_total % LT != 0 or nb > N_CORES):
        return _numpy_fallback(x, Wq, bq, Wk, bk, Wv, bv, Wo, bo)
    nc = _get_nc(l_total)
    in_maps = make_in_maps(x, Wq, Wk, Wv, Wo)
    res = run_bass_kernel_spmd(nc, in_maps, core_ids=list(range(nb)), trace=trace)
    out = np.stack([res.results[i]["out"] for i in range(nb)], axis=0)
    if trace:
        return out, res
    return out
